# revision 7
# baseline (speedup 1.0000x reference)
"""Differential attention (DiffAttn) kernel for 8 TRN2 NeuronCores.

Problem (hardcoded): B=2, N=2048, EMB=1024, H=8 heads, HD=64, LAMBDA_INIT=0.8.
 - qp/kp split into 2H=16 sub-heads of width 64; vp into H=8 heads of width 128.
 - attn = softmax(qh @ kh.T / 8); diff = attn[h,0] - lam * attn[h,1]; out = diff @ vh
 - out = attn_out @ Wo.T + bo

Sharding: 16 (batch, head-pair) units over 8 cores -> core c handles batch c//4
and heads {2*(c%4), 2*(c%4)+1} (a contiguous 256-column slice of the projection
space).  Each core computes its heads' attention output and a partial output
projection (column-parallel Wo); partials are summed on the host (no on-chip
collectives needed).  Biases: bq/bk applied on chip; bv is folded into the host
bias since rows of diff sum to exactly (1-lam); bo added on host.

On-chip layout avoids all activation transposes:
 - host feeds q.T/k.T/v.T so projections contract over EMB with e on partitions,
 - q/k projections are produced column-major [ch, n] (scores need d on partitions),
 - v projection is produced row-major [n, ch] (PV contraction needs k on partitions),
 - softmax rows live on partitions (free-dim sums via ACT accum_out),
 - P_diff is transposed with the DMA xbar (bf16) for the PV matmul.
"""

import math

import numpy as np
import ml_dtypes

BF = ml_dtypes.bfloat16

B, N, EMB = 2, 2048, 1024
H, HD = 8, 64
CH = 256          # channels per core = 2 heads
EBLK = 8          # EMB // 128 contraction blocks
QB = 16           # N // 128 query blocks
KB = 16           # N // 128 key blocks
NCH = 4           # N // 512 streaming chunks
LAMBDA_INIT = 0.8

_CACHE = {}


def _build(lam: float):
    import concourse.bass as bass
    import concourse.mybir as mybir
    import concourse.tile as tile
    from concourse import bacc

    dt = mybir.dt
    f32, bf = dt.float32, dt.bfloat16
    Alu = mybir.AluOpType
    Act = mybir.ActivationFunctionType
    MS = bass.MemorySpace
    nlam = -lam

    nc = bacc.Bacc("TRN2", target_bir_lowering=False, debug=False, num_devices=8)

    qT_d = nc.dram_tensor("qT", [EMB, N], bf, kind="ExternalInput")
    kT_d = nc.dram_tensor("kT", [EMB, N], bf, kind="ExternalInput")
    vT_d = nc.dram_tensor("vT", [EMB, N], bf, kind="ExternalInput")
    wq_d = nc.dram_tensor("wqT", [EMB, CH], bf, kind="ExternalInput")
    wk_d = nc.dram_tensor("wkT", [EMB, CH], bf, kind="ExternalInput")
    wv_d = nc.dram_tensor("wvT", [EMB, CH], bf, kind="ExternalInput")
    wo_d = nc.dram_tensor("woT", [CH, EMB], bf, kind="ExternalInput")
    bq_d = nc.dram_tensor("bq", [128, 2], f32, kind="ExternalInput")
    bk_d = nc.dram_tensor("bk", [128, 2], f32, kind="ExternalInput")
    out_d = nc.dram_tensor("out", [N, EMB], f32, kind="ExternalOutput")

    with tile.TileContext(nc) as tc:
        with (
            tc.tile_pool(name="wpool", bufs=1) as wpool,
            tc.tile_pool(name="instream", bufs=2) as instream,
            tc.tile_pool(name="persist", bufs=1) as persist,
            tc.tile_pool(name="epool", bufs=3) as epool,
            tc.tile_pool(name="pdpool", bufs=2) as pdpool,
            tc.tile_pool(name="small", bufs=4) as small,
            tc.tile_pool(name="outp", bufs=3) as outpool,
            tc.tile_pool(name="psS", bufs=2, space=MS.PSUM) as psS,
        ):
            wq_s = wpool.tile([128, EBLK, CH], bf, tag="wq")
            wk_s = wpool.tile([128, EBLK, CH], bf, tag="wk")
            wv_s = wpool.tile([128, EBLK, CH], bf, tag="wv")
            wo_s = wpool.tile([128, 2, EMB], bf, tag="wo")
            bq_s = wpool.tile([128, 2], f32, tag="bq")
            bk_s = wpool.tile([128, 2], f32, tag="bk")
            nc.sync.dma_start(wq_s[:], wq_d.ap().rearrange("(e p) c -> p e c", p=128))
            nc.sync.dma_start(wk_s[:], wk_d.ap().rearrange("(e p) c -> p e c", p=128))
            nc.sync.dma_start(wv_s[:], wv_d.ap().rearrange("(e p) c -> p e c", p=128))
            nc.sync.dma_start(wo_s[:], wo_d.ap().rearrange("(u p) c -> p u c", p=128))
            nc.sync.dma_start(bq_s[:], bq_d.ap())
            nc.sync.dma_start(bk_s[:], bk_d.ap())

            qhT = persist.tile([128, 2, N], bf, tag="qhT")   # [d|cb, unit, n]
            khT = persist.tile([128, 2, N], bf, tag="khT")
            V_s = persist.tile([128, KB, CH], bf, tag="V")   # [n_in_block, kb, ch]
            outT = persist.tile([128, 2, N], bf, tag="outT")  # [vd, unit, n]
            PdT = persist.tile([128, KB, N], bf, tag="PdT")  # [k_in_block, kb, q]

            # ---------------- projection helpers ----------------
            def proj_qk(t_d, w_s, b_s, dst, cb):
                src = t_d.ap().rearrange("(e p) n -> p e n", p=128)
                for nch in range(NCH):
                    chunk = instream.tile([128, EBLK, 512], bf, tag="chunk")
                    nc.sync.dma_start(chunk[:], src[:, :, nch * 512:(nch + 1) * 512])
                    psw = psS.tile([128, 2048], f32, tag="S")
                    ps = psw[:, 0:512]
                    for eb in range(EBLK):
                        nc.tensor.matmul(
                            ps[:],
                            w_s[:, eb, cb * 128:(cb + 1) * 128],
                            chunk[:, eb, :],
                            start=(eb == 0),
                            stop=(eb == EBLK - 1),
                        )
                    nc.vector.tensor_scalar(
                        dst[:, cb, nch * 512:(nch + 1) * 512],
                        ps[:], b_s[:, cb:cb + 1], None, Alu.add,
                    )

            def proj_v():
                vsrc = vT_d.ap().rearrange("(e p) n -> p e n", p=128)
                for nch in range(NCH):
                    chunk = instream.tile([128, EBLK, 512], bf, tag="chunk")
                    nc.sync.dma_start(chunk[:], vsrc[:, :, nch * 512:(nch + 1) * 512])
                    for nj in range(4):
                        nb = nch * 4 + nj
                        psw = psS.tile([128, 2048], f32, tag="S")
                        ps = psw[:, 0:512]
                        for eb in range(EBLK):
                            nc.tensor.matmul(
                                ps[:, 0:CH],
                                chunk[:, eb, nj * 128:(nj + 1) * 128],
                                wv_s[:, eb, :],
                                start=(eb == 0),
                                stop=(eb == EBLK - 1),
                            )
                        nc.vector.tensor_copy(V_s[:, nb, :], ps[:, 0:CH])

            # ---------------- attention ----------------
            GRP = 2  # query blocks per scalar-math group

            def attn_groups(u):
                for g in range(QB // GRP):
                    sp = small.tile([128, 2 * GRP], f32, tag="sp")
                    Es = []
                    for j in range(GRP):
                        qb = g * GRP + j
                        E = epool.tile([128, 2, N], bf, tag="E")
                        Es.append(E)
                        pss = [psS.tile([128, 2048], f32, tag="S", name=f"ps_sh{sh_}") for sh_ in range(2)]
                        lhs = [qhT[sh * 64:(sh + 1) * 64, u, qb * 128:(qb + 1) * 128]
                               for sh in range(2)]
                        for c4 in range(4):
                            for sh in range(2):
                                rhs = khT[sh * 64:(sh + 1) * 64, u,
                                          c4 * 512:(c4 + 1) * 512]
                                nc.tensor.matmul(pss[sh][:, c4 * 512:(c4 + 1) * 512],
                                                 lhs[sh], rhs, start=True, stop=True)
                        for sh in range(2):
                            t = j * 2 + sh
                            nc.scalar.activation(
                                E[:, sh, :], pss[sh][:],
                                Act.Exp, scale=0.125,
                                accum_out=sp[:, t:t + 1],
                            )
                    s1 = small.tile([128, GRP], f32, tag="s1")
                    r1 = small.tile([128, GRP], f32, tag="r1")
                    negc = small.tile([128, GRP], f32, tag="negc")
                    nc.vector.tensor_copy(s1[:], sp[:, 0::2])
                    nc.vector.tensor_copy(negc[:], sp[:, 1::2])
                    nc.vector.reciprocal(r1[:], s1[:])
                    nc.vector.reciprocal(negc[:], negc[:])                      # 1/s2
                    nc.vector.tensor_tensor(negc[:], negc[:], s1[:], Alu.mult)  # s1/s2
                    nc.vector.tensor_scalar(negc[:], negc[:], nlam, None, Alu.mult)
                    for j in range(GRP):
                        qb = g * GRP + j
                        E = Es[j]
                        Pd = pdpool.tile([128, N], bf, tag="Pd")
                        nc.vector.scalar_tensor_tensor(
                            Pd[:], E[:, 1, :], negc[:, j:j + 1], E[:, 0, :],
                            Alu.mult, Alu.add)
                        nc.vector.tensor_scalar(Pd[:], Pd[:], r1[:, j:j + 1], None, Alu.mult)
                        nc.sync.dma_start_transpose(PdT[:, :, qb * 128:(qb + 1) * 128], Pd[:])
            def pv(u):
                for qc in range(4):
                    psw = psS.tile([128, 2048], f32, tag="S")
                    ps = psw[:, 0:512]
                    for kb in range(KB):
                        nc.tensor.matmul(
                            ps[:],
                            V_s[:, kb, u * 128:(u + 1) * 128],
                            PdT[:, kb, qc * 512:(qc + 1) * 512],
                            start=(kb == 0),
                            stop=(kb == KB - 1),
                        )
                    nc.vector.tensor_copy(outT[:, u, qc * 512:(qc + 1) * 512], ps[:])

            # ---------------- phase schedule ----------------
            # k/q cb0 first -> unit0 attention can start; cb1 + v projections
            # fill the PE while unit0's exp stream runs on ACT.
            proj_qk(kT_d, wk_s, bk_s, khT, 0)
            proj_qk(qT_d, wq_s, bq_s, qhT, 0)
            attn_groups(0)
            proj_qk(kT_d, wk_s, bk_s, khT, 1)
            proj_qk(qT_d, wq_s, bq_s, qhT, 1)
            proj_v()
            pv(0)
            attn_groups(1)
            pv(1)

            # ---------------- output projection (partial: this core's 256 ch) ----
            for nb in range(QB):
                ob = outpool.tile([128, EMB], f32, tag="ob")
                for oc in range(2):
                    psw = psS.tile([128, 2048], f32, tag="S")
                    ps = psw[:, 0:512]
                    for cb in range(2):
                        nc.tensor.matmul(
                            ps[:],
                            outT[:, cb, nb * 128:(nb + 1) * 128],
                            wo_s[:, cb, oc * 512:(oc + 1) * 512],
                            start=(cb == 0),
                            stop=(cb == 1),
                        )
                    nc.vector.tensor_copy(ob[:, oc * 512:(oc + 1) * 512], ps[:])
                nc.sync.dma_start(out_d.ap()[nb * 128:(nb + 1) * 128, :], ob[:])

    nc.compile()
    return nc


def kernel(q, k, v, Wq, bq, Wk, bk, Wv, bv, Wo, bo,
           lambda_q1, lambda_k1, lambda_q2, lambda_k2, _trace=False):
    from concourse.bass_utils import run_bass_kernel_spmd

    q = np.asarray(q, dtype=np.float32)
    k = np.asarray(k, dtype=np.float32)
    v = np.asarray(v, dtype=np.float32)
    lam = float(np.exp(np.sum(np.float64(lambda_q1) * np.float64(lambda_k1)))
                - np.exp(np.sum(np.float64(lambda_q2) * np.float64(lambda_k2)))
                + LAMBDA_INIT)

    key = round(lam, 9)
    if key not in _CACHE:
        _CACHE.clear()
        _CACHE[key] = _build(lam)
    nc = _CACHE[key]

    qT = [np.ascontiguousarray(q[b].T).astype(BF) for b in range(B)]
    kT = [np.ascontiguousarray(k[b].T).astype(BF) for b in range(B)]
    vT = [np.ascontiguousarray(v[b].T).astype(BF) for b in range(B)]

    in_maps = []
    for c in range(8):
        b, hp = c // 4, c % 4
        cols = slice(CH * hp, CH * hp + CH)
        in_maps.append({
            "qT": qT[b], "kT": kT[b], "vT": vT[b],
            "wqT": np.ascontiguousarray(Wq[cols, :].T).astype(BF),
            "wkT": np.ascontiguousarray(Wk[cols, :].T).astype(BF),
            "wvT": np.ascontiguousarray(Wv[cols, :].T).astype(BF),
            "woT": np.ascontiguousarray(Wo[:, cols].T).astype(BF),
            "bq": np.ascontiguousarray(bq[cols].reshape(2, 128).T.astype(np.float32)),
            "bk": np.ascontiguousarray(bk[cols].reshape(2, 128).T.astype(np.float32)),
        })

    res = run_bass_kernel_spmd(nc, in_maps, core_ids=list(range(8)), trace=_trace)
    parts = [r["out"] for r in res.results]
    host_bias = (bo + (1.0 - lam) * (Wo @ bv)).astype(np.float32)
    out = np.stack([parts[0] + parts[1] + parts[2] + parts[3],
                    parts[4] + parts[5] + parts[6] + parts[7]])
    out += host_bias[None, None, :]
    if _trace:
        kernel._last_result = res
    return out.astype(np.float32)


# revision 8
# speedup vs baseline: 1.0998x; 1.0998x over previous
"""Differential attention (DiffAttn) kernel for 8 TRN2 NeuronCores.

Problem (hardcoded): B=2, N=2048, EMB=1024, H=8 heads, HD=64, LAMBDA_INIT=0.8.
 - qp/kp split into 2H=16 sub-heads of width 64; vp into H=8 heads of width 128.
 - attn = softmax(qh @ kh.T / 8); diff = attn[h,0] - lam * attn[h,1]; out = diff @ vh
 - out = attn_out @ Wo.T + bo

Sharding: 16 (batch, head-pair) units over 8 cores -> core c handles batch c//4
and heads {2*(c%4), 2*(c%4)+1} (a contiguous 256-column slice of the projection
space).  Each core computes its heads' attention output and a partial output
projection (column-parallel Wo); partials are summed on the host (no on-chip
collectives needed).  Biases: bq/bk applied on chip; bv is folded into the host
bias since rows of diff sum to exactly (1-lam); bo added on host.

On-chip layout avoids all activation transposes:
 - host feeds q.T/k.T/v.T so projections contract over EMB with e on partitions,
 - q/k projections are produced column-major [ch, n] (scores need d on partitions),
 - v projection is produced row-major [n, ch] (PV contraction needs k on partitions),
 - softmax rows live on partitions (free-dim sums via ACT accum_out),
 - P_diff is transposed with the DMA xbar (bf16) for the PV matmul.
"""

import math

import numpy as np
import ml_dtypes

BF = ml_dtypes.bfloat16

B, N, EMB = 2, 2048, 1024
H, HD = 8, 64
CH = 256          # channels per core = 2 heads
EBLK = 8          # EMB // 128 contraction blocks
QB = 16           # N // 128 query blocks
KB = 16           # N // 128 key blocks
NCH = 4           # N // 512 streaming chunks
LAMBDA_INIT = 0.8

_CACHE = {}


def _build(lam: float):
    import concourse.bass as bass
    import concourse.mybir as mybir
    import concourse.tile as tile
    from concourse import bacc

    dt = mybir.dt
    f32, bf = dt.float32, dt.bfloat16
    Alu = mybir.AluOpType
    Act = mybir.ActivationFunctionType
    MS = bass.MemorySpace
    nlam = -lam

    nc = bacc.Bacc("TRN2", target_bir_lowering=False, debug=False, num_devices=8)

    qT_d = nc.dram_tensor("qT", [EMB, N], bf, kind="ExternalInput")
    kT_d = nc.dram_tensor("kT", [EMB, N], bf, kind="ExternalInput")
    vT_d = nc.dram_tensor("vT", [EMB, N], bf, kind="ExternalInput")
    wq_d = nc.dram_tensor("wqT", [EMB, CH], bf, kind="ExternalInput")
    wk_d = nc.dram_tensor("wkT", [EMB, CH], bf, kind="ExternalInput")
    wv_d = nc.dram_tensor("wvT", [EMB, CH], bf, kind="ExternalInput")
    wo_d = nc.dram_tensor("woT", [CH, EMB], bf, kind="ExternalInput")
    bq_d = nc.dram_tensor("bq", [128, 2], f32, kind="ExternalInput")
    bk_d = nc.dram_tensor("bk", [128, 2], f32, kind="ExternalInput")
    out_d = nc.dram_tensor("out", [N, EMB], f32, kind="ExternalOutput")

    with tile.TileContext(nc) as tc:
        with (
            tc.tile_pool(name="wpool", bufs=1) as wpool,
            tc.tile_pool(name="instream", bufs=2) as instream,
            tc.tile_pool(name="persist", bufs=1) as persist,
            tc.tile_pool(name="epool", bufs=3) as epool,
            tc.tile_pool(name="pdpool", bufs=2) as pdpool,
            tc.tile_pool(name="small", bufs=4) as small,
            tc.tile_pool(name="outp", bufs=3) as outpool,
            tc.tile_pool(name="psS", bufs=2, space=MS.PSUM) as psS,
            tc.tile_pool(name="psM", bufs=3, space=MS.PSUM) as psM,
        ):
            wq_s = wpool.tile([128, EBLK, CH], bf, tag="wq")
            wk_s = wpool.tile([128, EBLK, CH], bf, tag="wk")
            wv_s = wpool.tile([128, EBLK, CH], bf, tag="wv")
            wo_s = wpool.tile([128, 2, EMB], bf, tag="wo")
            bq_s = wpool.tile([128, 2], f32, tag="bq")
            bk_s = wpool.tile([128, 2], f32, tag="bk")
            nc.sync.dma_start(wq_s[:], wq_d.ap().rearrange("(e p) c -> p e c", p=128))
            nc.sync.dma_start(wk_s[:], wk_d.ap().rearrange("(e p) c -> p e c", p=128))
            nc.sync.dma_start(wv_s[:], wv_d.ap().rearrange("(e p) c -> p e c", p=128))
            nc.sync.dma_start(wo_s[:], wo_d.ap().rearrange("(u p) c -> p u c", p=128))
            nc.sync.dma_start(bq_s[:], bq_d.ap())
            nc.sync.dma_start(bk_s[:], bk_d.ap())

            qhT = persist.tile([128, 2, N], bf, tag="qhT")   # [d|cb, unit, n]
            khT = persist.tile([128, 2, N], bf, tag="khT")
            V_s = persist.tile([128, KB, CH], bf, tag="V")   # [n_in_block, kb, ch]
            outT = persist.tile([128, 2, N], bf, tag="outT")  # [vd, unit, n]
            PdT = persist.tile([128, KB, N], bf, tag="PdT")  # [k_in_block, kb, q]

            # ---------------- projection helpers ----------------
            def proj_qk(t_d, w_s, b_s, dst, cb):
                src = t_d.ap().rearrange("(e p) n -> p e n", p=128)
                for nch in range(NCH):
                    chunk = instream.tile([128, EBLK, 512], bf, tag="chunk")
                    nc.sync.dma_start(chunk[:], src[:, :, nch * 512:(nch + 1) * 512])
                    ps = psM.tile([128, 512], f32, tag="mm")
                    for eb in range(EBLK):
                        nc.tensor.matmul(
                            ps[:],
                            w_s[:, eb, cb * 128:(cb + 1) * 128],
                            chunk[:, eb, :],
                            start=(eb == 0),
                            stop=(eb == EBLK - 1),
                        )
                    nc.vector.tensor_scalar(
                        dst[:, cb, nch * 512:(nch + 1) * 512],
                        ps[:], b_s[:, cb:cb + 1], None, Alu.add,
                    )

            def proj_v():
                vsrc = vT_d.ap().rearrange("(e p) n -> p e n", p=128)
                for nch in range(NCH):
                    chunk = instream.tile([128, EBLK, 512], bf, tag="chunk")
                    nc.sync.dma_start(chunk[:], vsrc[:, :, nch * 512:(nch + 1) * 512])
                    for nj in range(4):
                        nb = nch * 4 + nj
                        ps = psM.tile([128, 512], f32, tag="mm")
                        for eb in range(EBLK):
                            nc.tensor.matmul(
                                ps[:, 0:CH],
                                chunk[:, eb, nj * 128:(nj + 1) * 128],
                                wv_s[:, eb, :],
                                start=(eb == 0),
                                stop=(eb == EBLK - 1),
                            )
                        nc.vector.tensor_copy(V_s[:, nb, :], ps[:, 0:CH])

            # ---------------- attention ----------------
            GRP = 2  # query blocks per scalar-math group

            def attn_groups(u):
                for g in range(QB // GRP):
                    sp = small.tile([128, 4 * GRP], f32, tag="sp")
                    Es = []
                    for j in range(GRP):
                        qb = g * GRP + j
                        E = epool.tile([128, 2, N], bf, tag="E")
                        Es.append(E)
                        for sh in range(2):
                            lhsq = qhT[sh * 64:(sh + 1) * 64, u, qb * 128:(qb + 1) * 128]
                            for kh in range(2):
                                ps = psS.tile([128, 1024], f32, tag="S")
                                for c2 in range(2):
                                    rhs = khT[sh * 64:(sh + 1) * 64, u,
                                              kh * 1024 + c2 * 512: kh * 1024 + (c2 + 1) * 512]
                                    nc.tensor.matmul(ps[:, c2 * 512:(c2 + 1) * 512],
                                                     lhsq, rhs, start=True, stop=True)
                                t = j * 4 + sh * 2 + kh
                                nc.scalar.activation(
                                    E[:, sh, kh * 1024:(kh + 1) * 1024], ps[:],
                                    Act.Exp, scale=0.125,
                                    accum_out=sp[:, t:t + 1],
                                )
                    s1 = small.tile([128, GRP], f32, tag="s1")
                    r1 = small.tile([128, GRP], f32, tag="r1")
                    negc = small.tile([128, GRP], f32, tag="negc")
                    nc.vector.tensor_tensor(s1[:], sp[:, 0::4], sp[:, 1::4], Alu.add)
                    nc.vector.tensor_tensor(negc[:], sp[:, 2::4], sp[:, 3::4], Alu.add)
                    nc.vector.reciprocal(r1[:], s1[:])
                    nc.vector.reciprocal(negc[:], negc[:])                      # 1/s2
                    nc.vector.tensor_tensor(negc[:], negc[:], s1[:], Alu.mult)  # s1/s2
                    nc.vector.tensor_scalar(negc[:], negc[:], nlam, None, Alu.mult)
                    for j in range(GRP):
                        qb = g * GRP + j
                        E = Es[j]
                        Pd = pdpool.tile([128, N], bf, tag="Pd")
                        nc.vector.scalar_tensor_tensor(
                            Pd[:], E[:, 1, :], negc[:, j:j + 1], E[:, 0, :],
                            Alu.mult, Alu.add)
                        nc.vector.tensor_scalar(Pd[:], Pd[:], r1[:, j:j + 1], None, Alu.mult)
                        nc.sync.dma_start_transpose(PdT[:, :, qb * 128:(qb + 1) * 128], Pd[:])
            def pv(u):
                for qc in range(4):
                    ps = psM.tile([128, 512], f32, tag="mm")
                    for kb in range(KB):
                        nc.tensor.matmul(
                            ps[:],
                            V_s[:, kb, u * 128:(u + 1) * 128],
                            PdT[:, kb, qc * 512:(qc + 1) * 512],
                            start=(kb == 0),
                            stop=(kb == KB - 1),
                        )
                    nc.vector.tensor_copy(outT[:, u, qc * 512:(qc + 1) * 512], ps[:])

            # ---------------- phase schedule ----------------
            # k/q cb0 first -> unit0 attention can start; cb1 + v projections
            # fill the PE while unit0's exp stream runs on ACT.
            proj_qk(kT_d, wk_s, bk_s, khT, 0)
            proj_qk(qT_d, wq_s, bq_s, qhT, 0)
            attn_groups(0)
            proj_qk(kT_d, wk_s, bk_s, khT, 1)
            proj_qk(qT_d, wq_s, bq_s, qhT, 1)
            proj_v()
            pv(0)
            attn_groups(1)
            pv(1)

            # ---------------- output projection (partial: this core's 256 ch) ----
            for nb in range(QB):
                ob = outpool.tile([128, EMB], f32, tag="ob")
                for oc in range(2):
                    ps = psM.tile([128, 512], f32, tag="mm")
                    for cb in range(2):
                        nc.tensor.matmul(
                            ps[:],
                            outT[:, cb, nb * 128:(nb + 1) * 128],
                            wo_s[:, cb, oc * 512:(oc + 1) * 512],
                            start=(cb == 0),
                            stop=(cb == 1),
                        )
                    nc.vector.tensor_copy(ob[:, oc * 512:(oc + 1) * 512], ps[:])
                nc.sync.dma_start(out_d.ap()[nb * 128:(nb + 1) * 128, :], ob[:])

    nc.compile()
    return nc


def kernel(q, k, v, Wq, bq, Wk, bk, Wv, bv, Wo, bo,
           lambda_q1, lambda_k1, lambda_q2, lambda_k2, _trace=False):
    from concourse.bass_utils import run_bass_kernel_spmd

    q = np.asarray(q, dtype=np.float32)
    k = np.asarray(k, dtype=np.float32)
    v = np.asarray(v, dtype=np.float32)
    lam = float(np.exp(np.sum(np.float64(lambda_q1) * np.float64(lambda_k1)))
                - np.exp(np.sum(np.float64(lambda_q2) * np.float64(lambda_k2)))
                + LAMBDA_INIT)

    key = round(lam, 9)
    if key not in _CACHE:
        _CACHE.clear()
        _CACHE[key] = _build(lam)
    nc = _CACHE[key]

    qT = [np.ascontiguousarray(q[b].T).astype(BF) for b in range(B)]
    kT = [np.ascontiguousarray(k[b].T).astype(BF) for b in range(B)]
    vT = [np.ascontiguousarray(v[b].T).astype(BF) for b in range(B)]

    in_maps = []
    for c in range(8):
        b, hp = c // 4, c % 4
        cols = slice(CH * hp, CH * hp + CH)
        in_maps.append({
            "qT": qT[b], "kT": kT[b], "vT": vT[b],
            "wqT": np.ascontiguousarray(Wq[cols, :].T).astype(BF),
            "wkT": np.ascontiguousarray(Wk[cols, :].T).astype(BF),
            "wvT": np.ascontiguousarray(Wv[cols, :].T).astype(BF),
            "woT": np.ascontiguousarray(Wo[:, cols].T).astype(BF),
            "bq": np.ascontiguousarray(bq[cols].reshape(2, 128).T.astype(np.float32)),
            "bk": np.ascontiguousarray(bk[cols].reshape(2, 128).T.astype(np.float32)),
        })

    res = run_bass_kernel_spmd(nc, in_maps, core_ids=list(range(8)), trace=_trace)
    parts = [r["out"] for r in res.results]
    host_bias = (bo + (1.0 - lam) * (Wo @ bv)).astype(np.float32)
    out = np.stack([parts[0] + parts[1] + parts[2] + parts[3],
                    parts[4] + parts[5] + parts[6] + parts[7]])
    out += host_bias[None, None, :]
    if _trace:
        kernel._last_result = res
    return out.astype(np.float32)
